# revision 24
# baseline (speedup 1.0000x reference)
"""AttentiveReadout Trainium2 kernel (8-core SPMD, data-parallel over graphs).

Math per graph g (128 nodes each, nodes sorted by graph):
  scores[n,h] = x[n] @ Ws[:,h]        (Ws folds key_w & query; key_b shifts all
                                       scores in a segment equally so it drops
                                       out of the softmax; a 9th all-zero head
                                       yields e=1 -> exact mean-pool sums)
  e = exp(scores)                     (|s| < ~0.4, no max-shift needed)
  A[g]^T = x_g^T @ e_g                ([512, 9] per graph; col 8 = col-sums)
  Z[g,h] = sum_n e[n,h]               (ones^T @ e, on the PE)
  pooled = blockdiag_h(A/Z @ value_w.T); ctx = pooled @ out_w.T + ob2
  gate = sigmoid([ctx, avg] @ gate_w.T + gb2);  avg = A[g,:,8]/128
  out = LayerNorm(avg + gate*(ctx-avg)) * ln_w + ln_b

v2 layout: everything node-major. x ships in TWO stationary layouts:
node-major bf16 (A-matmul lhsT) and feature-major fp8-e3m4 (scores lhsT,
errors there are suppressed by the softmax and 128-node averaging).  No
on-chip transposes in phase 1, exp works on [128, 144] tiles, Z comes from
a 1-column ones matmul instead of a DVE reduction.  Phase-2 matmuls all in
bf16.  DMA: 2 MB / 1 MB HWDGE loads (SP engine), 8 blocks of 16 graphs.
"""
import numpy as np
import ml_dtypes

N_NODES = 131072
IN_F = 512
H = 8
DH = 64
B = 1024
SEG = 128          # nodes per graph
N_CORES = 8
GPC = B // N_CORES     # graphs per core = 128
NPC = N_NODES // N_CORES   # nodes per core = 16384
NB = 16            # graphs per block
LN_EPS = 1e-5

_CACHE = {}
_P2SPLIT = True
_ABLATE = set()  # ablation flags (profiling only)


def _build_nc(G, repeat=1):
    """Build the single-core Bass program for G graphs (NB graphs per block).

    repeat>1 wraps the whole body in a For_i that re-runs it (identical
    output) — used only for timing, to amortize the dispatch overhead."""
    import concourse.mybir as mybir
    import concourse.tile as tile
    import concourse.bacc as bacc

    f32 = mybir.dt.float32
    bf16 = mybir.dt.bfloat16
    f8 = mybir.dt.float8e3
    AFT = mybir.ActivationFunctionType

    NBLK = G // NB

    nc = bacc.Bacc("TRN2", target_bir_lowering=False, debug=False,
                   num_devices=N_CORES)
    # one merged stream per block: cols [0, NB*512) node-major (A lhsT),
    # cols [NB*512, 2*NB*512) feature-major (scores lhsT)
    xall = nc.declare_dram_parameter("xall", [NBLK, 128, 2 * NB * 512], f8,
                                     isOutput=False)
    ws = nc.declare_dram_parameter("ws", [128, 36], bf16, isOutput=False)
    ones128 = nc.declare_dram_parameter("ones128", [128, 1], bf16,
                                        isOutput=False)
    ones64 = nc.declare_dram_parameter("ones64", [1, 64], bf16, isOutput=False)
    ihalf = nc.declare_dram_parameter("ihalf", [128, 128], f32, isOutput=False)
    vw = nc.declare_dram_parameter("vw", [128, 2048], bf16, isOutput=False)
    ow = nc.declare_dram_parameter("ow", [128, 2048], bf16, isOutput=False)
    gw = nc.declare_dram_parameter("gw", [128, 4096], bf16, isOutput=False)
    ob2 = nc.declare_dram_parameter("ob2", [128, 4], f32, isOutput=False)
    gb2 = nc.declare_dram_parameter("gb2", [128, 4], f32, isOutput=False)
    lnw = nc.declare_dram_parameter("lnw", [128, 512], f32, isOutput=False)
    lnb = nc.declare_dram_parameter("lnb", [128, 512], f32, isOutput=False)
    out = nc.declare_dram_parameter("out", [G, 512], f32, isOutput=True)

    with tile.TileContext(nc) as tc:
        from contextlib import ExitStack
        with ExitStack() as octx:
            if repeat > 1:
                octx.enter_context(tc.For_i(
                    0, repeat, 1,
                    hint_engines=(mybir.EngineType.PE, mybir.EngineType.DVE,
                                  mybir.EngineType.Activation,
                                  mybir.EngineType.Pool, mybir.EngineType.SP)))
            ctx = octx
            cpool = ctx.enter_context(tc.tile_pool(name="const", bufs=1))
            ws_sb = cpool.tile([128, 36], bf16, tag="ws")
            nc.scalar.dma_start(ws_sb[:], ws[:])
            o128_sb = cpool.tile([128, 1], bf16, tag="o128")
            nc.scalar.dma_start(o128_sb[:], ones128[:])
            # phase-2 weights: issue at t=0 on the idle SWDGE queue so they
            # trickle in alongside the x stream and are resident by phase 2.
            # Small constants first so nothing cheap queues behind megabytes.
            p2w = ctx.enter_context(tc.tile_pool(name="p2w", bufs=1))
            o64_sb = p2w.tile([1, 64], bf16, tag="o64")
            nc.gpsimd.dma_start(o64_sb[:], ones64[:])
            ob2_sb = p2w.tile([128, 4], f32, tag="ob2")
            nc.gpsimd.dma_start(ob2_sb[:], ob2[:])
            gb2_sb = p2w.tile([128, 4], f32, tag="gb2")
            nc.gpsimd.dma_start(gb2_sb[:], gb2[:])
            ih_sb = p2w.tile([128, 128], f32, tag="ih")
            nc.gpsimd.dma_start(ih_sb[:], ihalf[:])
            lw_b = p2w.tile([128, 512], f32, tag="lwb")
            nc.gpsimd.dma_start(lw_b[:], lnw[:])
            lb_b = p2w.tile([128, 512], f32, tag="lbb")
            nc.gpsimd.dma_start(lb_b[:], lnb[:])
            vw_sb = p2w.tile([128, 2048], bf16, tag="vw")
            nc.gpsimd.dma_start(vw_sb[:], vw[:])
            ow_sb = p2w.tile([128, 2048], bf16, tag="ow")
            nc.gpsimd.dma_start(ow_sb[:], ow[:])
            gw_sb = p2w.tile([128, 4096], bf16, tag="gw")
            nc.gpsimd.dma_start(gw_sb[:], gw[:])

            apool = ctx.enter_context(tc.tile_pool(name="accum", bufs=1))
            # A^T for all graphs: column g*36 + fb*9 + h  (f = fb*128 + p)
            at_sb = apool.tile([128, G * 36], bf16, tag="at")
            z_sb = apool.tile([1, G * 9], f32, tag="z")

            # ---------------- phase 1: per-graph sums ----------------
            QO = NB * 512  # column offset of the feature-major half

            def emit_phase2(p2, p2ps, g0, ng):
                """Readout for graphs [g0, g0+ng): needs z_sb/at_sb for that
                range only, so the first half can run under late phase-1."""
                t = lambda s: f"{s}_{g0}"
                rz_f = p2.tile([1, ng * 9], f32, tag=t("rzf"))
                nc.vector.reciprocal(rz_f[:], z_sb[:, g0 * 9:(g0 + ng) * 9])
                rz = p2.tile([1, ng * 9], bf16, tag=t("rz"))
                nc.vector.tensor_copy(rz[:], rz_f[:])
                rz_r = rz[:].rearrange("p (blk b w) -> p blk b w", b=4, w=9)
                rzp = p2ps.tile([128, 4 * ng], f32, tag=t("rzp"))
                for pb in range(4):
                    for half in range(2):
                        h0 = 2 * pb + half
                        nc.tensor.matmul(
                            rzp[64 * half:64 * half + 64, pb * ng:(pb + 1) * ng],
                            o64_sb[:], rz_r[:, :, :, h0], start=True, stop=True)
                rz_big = p2.tile([128, 4 * ng], bf16, tag=t("rzbig"))
                nc.vector.tensor_copy(rz_big[:], rzp[:])

                # pooled^T [c, g] (unnormalized), then normalize by rz_big
                pooled = p2.tile([128, 4 * ng], bf16, tag=t("pooled"))
                for h in range(8):
                    pps = p2ps.tile([64, ng], f32, tag=t("pps"))
                    for kc in range(4):
                        nc.tensor.matmul(
                            pps[:],
                            vw_sb[:, kc * 512 + h * 64: kc * 512 + (h + 1) * 64],
                            at_r[:, kc * 9 + h, g0:g0 + ng],
                            start=(kc == 0), stop=(kc == 3))
                    hp, base = h // 2, (h % 2) * 64
                    nc.vector.tensor_copy(
                        pooled[base:base + 64, hp * ng:(hp + 1) * ng], pps[:])
                nc.vector.tensor_mul(pooled[:], pooled[:], rz_big[:])

                # avg^T (normalized by exact 1/128)
                avgn = p2.tile([128, 4 * ng], bf16, tag=t("avgn"))
                for pb in range(4):
                    nc.scalar.mul(avgn[:, pb * ng:(pb + 1) * ng],
                                  at_r[:, pb * 9 + 8, g0:g0 + ng], 1.0 / SEG)

                # ctx^T = out_w @ pooled^T + ob2
                ctxt = p2.tile([128, 4 * ng], bf16, tag=t("ctxt"))
                for mb in range(4):
                    cps = p2ps.tile([128, ng], f32, tag=t("cps"))
                    for kc in range(4):
                        nc.tensor.matmul(
                            cps[:],
                            ow_sb[:, kc * 512 + mb * 128: kc * 512 + (mb + 1) * 128],
                            pooled[:, kc * ng:(kc + 1) * ng],
                            start=(kc == 0), stop=(kc == 3))
                    nc.scalar.activation(ctxt[:, mb * ng:(mb + 1) * ng], cps[:],
                                         AFT.Identity, bias=ob2_sb[:, mb:mb + 1])

                # gate = sigmoid(u) = 0.5 + 0.5*tanh(u/2): tanh shares the
                # exp act-table set (sigmoid would force a 3rd table load);
                # gb2 holds gate_b/2.  avg-side chunks accumulate first so
                # they don't wait on ctxt.
                gate = p2.tile([128, 4 * ng], bf16, tag=t("gate"))
                for mb in range(4):
                    gps = p2ps.tile([128, ng], f32, tag=t("gps"))
                    for i, kc in enumerate((4, 5, 6, 7, 0, 1, 2, 3)):
                        rhs = (ctxt[:, kc * ng:(kc + 1) * ng] if kc < 4
                               else avgn[:, (kc - 4) * ng:(kc - 3) * ng])
                        nc.tensor.matmul(
                            gps[:],
                            gw_sb[:, kc * 512 + mb * 128: kc * 512 + (mb + 1) * 128],
                            rhs, start=(i == 0), stop=(i == 7))
                    nc.scalar.activation(gate[:, mb * ng:(mb + 1) * ng], gps[:],
                                         AFT.Tanh, bias=gb2_sb[:, mb:mb + 1],
                                         scale=0.5)

                # emb = avg + sigmoid(u)*(ctx-avg) = 0.5*((ctx+avg)+t*(ctx-avg));
                # the 0.5 is folded into the transpose identity (ihalf)
                d = p2.tile([128, 4 * ng], bf16, tag=t("d"))
                nc.vector.tensor_sub(d[:], ctxt[:], avgn[:])
                nc.vector.tensor_mul(d[:], gate[:], d[:])
                m = p2.tile([128, 4 * ng], bf16, tag=t("m"))
                nc.vector.tensor_add(m[:], ctxt[:], avgn[:])
                embt = p2.tile([128, 4 * ng], f32, tag=t("embt"))
                nc.vector.tensor_add(embt[:], m[:], d[:])

                # transpose (and halve) emb^T -> emb [ng, 512]
                emb = p2.tile([ng, 512], f32, tag=t("emb"))
                for pb in range(4):
                    tps = p2ps.tile([ng, 128], f32, tag=t("tps"))
                    nc.tensor.transpose(tps[:], embt[:, pb * ng:(pb + 1) * ng],
                                        ih_sb[:])
                    nc.vector.tensor_copy(emb[:, pb * 128:(pb + 1) * 128], tps[:])

                # LayerNorm over features: fused mean/var on the DVE
                stats = p2.tile([ng, 6], f32, tag=t("stats"))
                nc.vector.bn_stats(stats[:], emb[:])
                mv = p2.tile([ng, 2], f32, tag=t("mv"))
                nc.vector.bn_aggr(mv[:], stats[:])
                cent = p2.tile([ng, 512], f32, tag=t("cent"))
                nc.vector.tensor_scalar_sub(cent[:], emb[:], mv[:, 0:1])
                eps = p2.tile([ng, 1], f32, tag=t("eps"))
                nc.vector.memset(eps[:], LN_EPS)
                sd = p2.tile([ng, 1], f32, tag=t("sd"))
                nc.scalar.activation(sd[:], mv[:, 1:2], AFT.Sqrt, bias=eps[:])
                rstd = p2.tile([ng, 1], f32, tag=t("rstd"))
                nc.vector.reciprocal(rstd[:], sd[:])

                res = p2.tile([ng, 512], f32, tag=t("res"))
                nc.vector.scalar_tensor_tensor(
                    res[:], cent[:], rstd[:], lw_b[0:ng, :],
                    mybir.AluOpType.mult, mybir.AluOpType.mult)
                nc.vector.tensor_add(res[:], res[:], lb_b[0:ng, :])
                nc.sync.dma_start(out[g0:g0 + ng], res[:])

            at_r = at_sb[:].rearrange("p (g w) -> p w g", w=36)
            with tc.tile_pool(name="xin", bufs=(4 if NB <= 16 else 2)) as xpool, \
                 tc.tile_pool(name="esb", bufs=3) as epool, \
                 tc.tile_pool(name="p2", bufs=1) as p2, \
                 tc.tile_pool(name="sps", bufs=2, space="PSUM") as spspool, \
                 tc.tile_pool(name="zps", bufs=1, space="PSUM") as zpspool, \
                 tc.tile_pool(name="aps", bufs=2, space="PSUM") as apspool, \
                 tc.tile_pool(name="p2ps", bufs=3, space="PSUM") as p2ps:
                for blk in range(NBLK):
                    xt = xpool.tile([128, 2 * NB * 512], f8, tag="xall")
                    if "tinydma" in _ABLATE:
                        nc.sync.dma_start(xt[:, 0:8], xall[0, :, 0:8])
                    else:
                        nc.sync.dma_start(xt[:], xall[blk])

                    # scores, node-major: out [128 nodes, 9] per graph
                    sps = spspool.tile([128, NB * 9], f32, tag="sps")
                    for b in range(NB):
                        nfb = 1 if "noscores" in _ABLATE else 4
                        for fb in range(nfb):
                            nc.tensor.matmul(
                                sps[:, b * 9:(b + 1) * 9],
                                xt[:, QO + b * 512 + fb * 128:
                                   QO + b * 512 + (fb + 1) * 128],
                                ws_sb[:, fb * 9:(fb + 1) * 9],
                                start=(fb == 0), stop=(fb == nfb - 1))
                    esb = epool.tile([128, NB * 9], bf16, tag="e")
                    nc.scalar.activation(esb[:], sps[:], AFT.Exp)
                    # Z[g,h] via ones-matmul
                    zp = zpspool.tile([1, NB * 9], f32, tag="zp")
                    nc.tensor.matmul(zp[:], o128_sb[:], esb[:],
                                     start=True, stop=True)
                    nc.vector.tensor_copy(
                        z_sb[:, blk * NB * 9:(blk + 1) * NB * 9], zp[:])
                    # A^T: per (graph, f-block) matmul, 8 graphs per PSUM tile
                    for q in range(NB // 8):
                        aps = apspool.tile([128, 8 * 36], f32, tag="aps")
                        for b8 in range(8):
                            b = q * 8 + b8
                            nfb = 1 if "noamm" in _ABLATE else 4
                            for fb in range(nfb):
                                nc.tensor.matmul(
                                    aps[:, b8 * 36 + fb * 9:
                                        b8 * 36 + (fb + 1) * 9],
                                    xt[:, b * 512 + fb * 128:
                                       b * 512 + (fb + 1) * 128],
                                    esb[:, b * 9:(b + 1) * 9],
                                    start=(fb == 0), stop=(fb == nfb - 1))
                        nc.vector.tensor_copy(
                            at_sb[:, (blk * (NB // 8) + q) * 288:
                                  (blk * (NB // 8) + q + 1) * 288], aps[:])
                    if _P2SPLIT and blk == NBLK // 2 - 1:
                        emit_phase2(p2, p2ps, 0, G // 2)
                if _P2SPLIT:
                    emit_phase2(p2, p2ps, G // 2, G // 2)
                else:
                    emit_phase2(p2, p2ps, 0, G)
    nc.compile()
    return nc


def _prep_weights(query, key_w, value_w, out_w, out_b, value_b, gate_w,
                  gate_b, ln_w, ln_b):
    bf16 = ml_dtypes.bfloat16
    F = IN_F
    # scores weight: Ws[f,h] = sum_d key_w[h*64+d, f]*query[h,d]; col 8 = 0
    Ws = np.zeros((F, 9), np.float32)
    Ws[:, :H] = (key_w.reshape(H, DH, F) * query[:, :, None]).sum(1).T
    ws_r = np.zeros((128, 36), np.float32)
    for fb in range(4):
        ws_r[:, fb * 9:(fb + 1) * 9] = Ws[fb * 128:(fb + 1) * 128, :]
    com = {
        "ws": ws_r.astype(bf16),
        "ones128": np.ones((128, 1), np.float32).astype(bf16),
        "ones64": np.ones((1, 64), np.float32).astype(bf16),
        "ihalf": 0.5 * np.eye(128, dtype=np.float32),
        "vw": np.ascontiguousarray(
            value_w.T.reshape(4, 128, 512).transpose(1, 0, 2)
            .reshape(128, 2048)).astype(bf16),
        "ow": np.ascontiguousarray(
            out_w.T.reshape(4, 128, 512).transpose(1, 0, 2)
            .reshape(128, 2048)).astype(bf16),
        "gw": np.ascontiguousarray(
            gate_w.T.reshape(8, 128, 512).transpose(1, 0, 2)
            .reshape(128, 4096)).astype(bf16),
        "ob2": np.ascontiguousarray((out_b + out_w @ value_b).reshape(4, 128).T),
        "gb2": np.ascontiguousarray(0.5 * gate_b.reshape(4, 128).T),
        "lnw": np.tile(ln_w.astype(np.float32).reshape(1, 512), (128, 1)),
        "lnb": np.tile(ln_b.astype(np.float32).reshape(1, 512), (128, 1)),
    }
    return {k: np.ascontiguousarray(v) for k, v in com.items()}


def _reference_np(x, batch, query, key_w, key_b, value_w, value_b, out_w,
                  out_b, gate_w, gate_b, ln_w, ln_b):
    """Safety-net numpy fallback for unexpected (non-uniform) batch layouts."""
    N = x.shape[0]
    nb = int(batch.max()) + 1
    keys = (x @ key_w.T + key_b).reshape(N, H, DH)
    scores = np.einsum('nhd,hd->nh', keys, query)
    smax = np.full((nb, H), -np.inf, np.float32)
    np.maximum.at(smax, batch, scores)
    smax = np.where(np.isfinite(smax), smax, 0.0)
    e = np.exp(scores - smax[batch])
    ssum = np.zeros((nb, H), np.float32)
    np.add.at(ssum, batch, e)
    w = e / np.maximum(ssum[batch], 1e-12)
    values = (x @ value_w.T + value_b).reshape(N, H, DH)
    pooled = np.zeros((nb, H, DH), np.float32)
    np.add.at(pooled, batch, w[:, :, None] * values)
    ctx = pooled.reshape(nb, H * DH) @ out_w.T + out_b
    counts = np.zeros((nb,), np.float32)
    np.add.at(counts, batch, np.ones((N,), np.float32))
    avg = np.zeros((nb, x.shape[1]), np.float32)
    np.add.at(avg, batch, x)
    avg = avg / np.maximum(counts, 1.0)[:, None]
    gate = 1.0 / (1.0 + np.exp(-(np.concatenate([ctx, avg], 1) @ gate_w.T + gate_b)))
    ctx = gate * ctx + (1.0 - gate) * avg
    emb = np.where(counts[:, None] > 0, ctx, 0.0)
    mu = emb.mean(-1, keepdims=True)
    var = emb.var(-1, keepdims=True)
    return ((emb - mu) / np.sqrt(var + LN_EPS) * ln_w + ln_b).astype(np.float32)


def _fp8_colsum_preserving(xg):
    """Quantize [NG, 128, 512] to fp8-e3m4 with error-feedback rounding down
    the node axis: each element is still an fp8 value within one quantization
    step of its input, but per-(graph, feature) column sums are preserved to
    ~a single element's rounding error (the mean-pool path needs this)."""
    f8 = ml_dtypes.float8_e3m4
    out = np.empty(xg.shape, f8)
    c = np.zeros((xg.shape[0], xg.shape[2]), np.float32)
    for n in range(xg.shape[1]):
        t = xg[:, n, :] + c
        q = t.astype(f8)
        out[:, n, :] = q
        c = t - q.astype(np.float32)
    return out


def _prearrange(x_shard):
    """Pre-tile a per-core [NPC, 512] fp32 shard into one merged fp8 stream:
    cols [0, NB*512):         xall[blk, p, b*512+f]      = fp8_ef(x[(blk*NB+b)*128+p, f])
    cols [NB*512, 2*NB*512):  xall[blk, p, QO+b*512+fb*128+n] = fp8(x[(blk*NB+b)*128+n, fb*128+p])"""
    NN = x_shard.shape[0]
    NBLK = NN // (NB * SEG)
    xb8 = _fp8_colsum_preserving(x_shard.reshape(NN // SEG, SEG, 512))
    xb = (xb8.reshape(NBLK, NB, 128, 512).transpose(0, 2, 1, 3)
          .reshape(NBLK, 128, NB * 512))
    x8 = x_shard.astype(ml_dtypes.float8_e3m4)
    xq = (x8.reshape(NBLK, NB, 128, 4, 128).transpose(0, 4, 1, 3, 2)
          .reshape(NBLK, 128, NB * 512))
    return np.ascontiguousarray(np.concatenate([xb, xq], axis=2))


def _make_inmaps(x, query, key_w, value_w, value_b, out_w, out_b, gate_w,
                 gate_b, ln_w, ln_b):
    com = _prep_weights(query, key_w, value_w, out_w, out_b, value_b, gate_w,
                        gate_b, ln_w, ln_b)
    in_maps = []
    for k in range(N_CORES):
        m = dict(com)
        m["xall"] = _prearrange(x[k * NPC:(k + 1) * NPC])
        in_maps.append(m)
    return in_maps


def _make_exec_fn(nc, in_maps):
    """Build a non-donating jitted executor over 8 cores with device-resident
    inputs.  Returns (fn, dev_args)."""
    import jax
    import numpy as np
    from jax.sharding import Mesh, PartitionSpec, NamedSharding
    from jax.experimental.shard_map import shard_map
    from concourse import bass2jax, mybir

    part_name = (nc.partition_id_tensor.name
                 if nc.partition_id_tensor else None)
    in_names, out_names, out_avals, zero_outs = [], [], [], []
    for alloc in nc.m.functions[0].allocations:
        if not isinstance(alloc, mybir.MemoryLocationSet):
            continue
        name = alloc.memorylocations[0].name
        if alloc.kind == "ExternalInput":
            if name != part_name:
                in_names.append(name)
        elif alloc.kind == "ExternalOutput":
            out_names.append(name)
            dt_np = mybir.dt.np(alloc.dtype)
            out_avals.append(jax.core.ShapedArray(
                tuple(alloc.tensor_shape), dt_np))
            zero_outs.append(np.zeros(tuple(alloc.tensor_shape), dt_np))
    n_params = len(in_names)
    all_in_names = list(in_names) + list(out_names)
    if part_name is not None:
        all_in_names.append(part_name)

    def _body(*params):
        operands = list(params)
        if part_name is not None:
            operands.append(bass2jax.partition_id_tensor())
        outs = bass2jax._bass_exec_p.bind(
            *operands,
            out_avals=tuple(out_avals),
            in_names=tuple(all_in_names),
            out_names=tuple(out_names),
            lowering_input_output_aliases=(),
            sim_require_finite=True,
            sim_require_nnan=True,
            nc=nc)
        return tuple(outs)

    devices = jax.devices()[:N_CORES]
    mesh = Mesh(np.array(devices), ("core",))
    spec = PartitionSpec("core")
    n_outs = len(out_avals)
    fn = jax.jit(shard_map(_body, mesh=mesh,
                           in_specs=(spec,) * (n_params + n_outs),
                           out_specs=(spec,) * n_outs, check_rep=False),
                 keep_unused=True)
    sh = NamedSharding(mesh, spec)
    dev_args = [jax.device_put(
                    np.concatenate([np.asarray(m[nm]) for m in in_maps], 0), sh)
                for nm in in_names]
    dev_args += [jax.device_put(
                    np.zeros((N_CORES * z.shape[0], *z.shape[1:]), z.dtype), sh)
                 for z in zero_outs]
    return fn, dev_args


def _time_exec(fn, dev_args, reps):
    import jax, time
    outs = fn(*dev_args)
    jax.block_until_ready(outs)
    best = float("inf")
    for _ in range(3):
        t0 = time.perf_counter()
        res = [fn(*dev_args) for _ in range(reps)]
        jax.block_until_ready(res)
        best = min(best, (time.perf_counter() - t0) / reps)
    return best


def profile_hw_ns(inputs, r_lo=8, r_hi=136):
    """True per-execution HW time via repeat-loop slope: build the kernel
    wrapped in a For_i that runs the body R times per dispatch, measure wall
    time at two R values through identical dispatch paths, and difference.
    Returns ns per kernel body execution (includes ~6us loop back-edge)."""
    try:
        import numpy as np
        args = [np.asarray(inputs[k], np.float32) for k in
                ("query", "key_w", "value_w", "value_b", "out_w", "out_b",
                 "gate_w", "gate_b", "ln_w", "ln_b")]
        (query, key_w, value_w, value_b, out_w, out_b, gate_w, gate_b,
         ln_w, ln_b) = args
        in_maps = _make_inmaps(np.asarray(inputs["x"], np.float32), query,
                               key_w, value_w, value_b, out_w, out_b,
                               gate_w, gate_b, ln_w, ln_b)
        ts = {}
        for r in (r_lo, r_hi):
            key = f"nc_rep{r}"
            nc = _CACHE.get(key)
            if nc is None:
                nc = _CACHE[key] = _build_nc(GPC, repeat=r)
            fn, dev_args = _make_exec_fn(nc, in_maps)
            ts[r] = _time_exec(fn, dev_args, reps=8)
            print(f"[profile] R={r}: wall/call={ts[r]*1e6:.1f}us")
        return int((ts[r_hi] - ts[r_lo]) / (r_hi - r_lo) * 1e9)
    except Exception:
        import traceback
        traceback.print_exc()
        return None


def kernel(x, batch, query, key_w, key_b, value_w, value_b, out_w, out_b,
           gate_w, gate_b, ln_w, ln_b):
    x = np.asarray(x, np.float32)
    batch = np.asarray(batch)
    args = [np.asarray(a, np.float32) for a in
            (query, key_w, key_b, value_w, value_b, out_w, out_b, gate_w,
             gate_b, ln_w, ln_b)]
    (query, key_w, key_b, value_w, value_b, out_w, out_b, gate_w, gate_b,
     ln_w, ln_b) = args

    exp_batch = (np.arange(N_NODES) // SEG).astype(batch.dtype)
    if x.shape != (N_NODES, IN_F) or not np.array_equal(batch, exp_batch):
        return _reference_np(x, batch, query, key_w, key_b, value_w, value_b,
                             out_w, out_b, gate_w, gate_b, ln_w, ln_b)

    from concourse.bass_utils import run_bass_kernel_spmd

    if "nc" not in _CACHE:
        _CACHE["nc"] = _build_nc(GPC)
    nc = _CACHE["nc"]

    in_maps = _make_inmaps(x, query, key_w, value_w, value_b, out_w, out_b,
                           gate_w, gate_b, ln_w, ln_b)
    res = run_bass_kernel_spmd(nc, in_maps, list(range(N_CORES)))
    return np.concatenate([res.results[k]["out"] for k in range(N_CORES)], 0)
